# revision 62
# baseline (speedup 1.0000x reference)
"""Multi-head causal attention (B=4, S=2048, D=1024, H=16, dk=dv=64) on 8 NeuronCores.

Sharding: core c -> (batch b = c//2, head-group g = c%2 of 8 heads).
Each core computes Q/K/V projections for its batch restricted to its 8 heads,
causal softmax attention, and a partial output projection with its 512 rows of
Wo.  The host sums the two partials per batch and adds the constant correction
bv @ Wo + bo (bv passes through attention linearly because softmax rows sum
to 1).

Dtype strategy (keyed to the TRN2 matmul cost model: cost = out_free x
cycles_per_row; fp8 DoubleRow = 0.5 c/r with 2x128 contraction per
instruction, everything else 1.0):
  - QKV projections: x and W shipped from host as same-scale fp8e4 hi/lo
    pairs; 3-term (hi.hi + lo.hi + hi.lo) DoubleRow accumulation -> 0.75
    cycles per 128-contraction chunk instead of 1.0, with ~bf16 accuracy.
  - Scores: Q^T/K^T evicted to fp8e4 (x2 / x16 scales folded host-side)
    stored as [*, 2, S] with a zero second block, so a single DoubleRow
    matmul with d_k=64 contraction costs 0.5 c/r (the zero block contributes
    nothing and is free).
  - exp on ACT with scale=1/256 folding the score descale; bf16 P out.
  - AV v-major: au^T[65, 512] per (h, q-tile) with the ones row of V' as
    softmax denominator; bf16 rhs = exp output.  Big 512-col matmuls keep
    the PE sequencer (131 ns per Ldweights+Matmult pair in the cost model)
    off the critical path.
  - Normalization: reciprocal_approx_fast on the denominator row, broadcast
    across 64 partitions with a tiny f32r matmul, one tensor_tensor multiply
    into bf16 A^T.
  - Output projection and DMA in bf16.

Engine budget per core: ACT (exp, ~143us) is the bottleneck; PE ~155us of
issue interleaved so scores start ~10us in; DVE/Pool carry evictions,
reciprocals, masks and copies.
"""

import numpy as np
import ml_dtypes
from contextlib import ExitStack

import concourse.bass as bass
import concourse.mybir as mybir
import concourse.tile as tile
from concourse import bacc, bass_utils

N_HEAD, D_MODEL, D_K, D_V = 16, 1024, 64, 64
BATCH, SEQ = 4, 2048
NCORES = 8
S = SEQ
DM = D_MODEL
HV = 8 * D_V          # 512 local head-value columns per core
KC = DM // 128        # 8 d_model chunks
NPAIR = 4             # local head pairs
NQT = S // 512        # 4 q-tiles
F32 = mybir.dt.float32
BF16 = mybir.dt.bfloat16
F8 = mybir.dt.float8e4

SX = 16.0             # fp8 scale for x (hi and lo use the same scale)
SW = 8.0              # fp8 scale for projection weights
SQ = 16.0             # extra scale on the Q/K paths so fp8 eviction is exact
EVS = 1.0 / (SX * SW)          # psum -> Q/K/V descale
EXPS = 1.0 / (2.0 * SQ * 8.0)  # q8*k8 -> exp argument (incl. 1/sqrt(dk))

_CACHED_NC = None


def _build_nc(nbody=1, phases="ABC"):
    nc = bacc.Bacc("TRN2", target_bir_lowering=False, debug=False)

    x8 = nc.dram_tensor("x8", [DM, 2, S], F8, kind="ExternalInput").ap()
    wqh = nc.dram_tensor("wqh", [DM, HV], F8, kind="ExternalInput").ap()
    wkh = nc.dram_tensor("wkh", [DM, HV], F8, kind="ExternalInput").ap()
    wvh = nc.dram_tensor("wvh", [DM, HV], F8, kind="ExternalInput").ap()
    wvl = nc.dram_tensor("wvl", [DM, HV], F8, kind="ExternalInput").ap()
    wo = nc.dram_tensor("wo", [HV, DM], BF16, kind="ExternalInput").ap()
    bq = nc.dram_tensor("bq", [HV], F32, kind="ExternalInput").ap()
    bk = nc.dram_tensor("bk", [HV], F32, kind="ExternalInput").ap()
    masks = nc.dram_tensor("masks", [128, 128], BF16, kind="ExternalInput").ap()
    o = nc.dram_tensor("o", [S, DM], BF16, kind="ExternalOutput").ap()

    with tile.TileContext(nc) as tc:
        for _ in range(nbody):
            _build_kernel(tc, nc, x8, wqh, wkh, wvh, wvl,
                          wo, bq, bk, masks, o)
    nc.compile()
    return nc


def _build_kernel(tc, nc, x8, wqh, wkh, wvh, wvl,
                  wo, bq, bk, masks, o):
    EXP = mybir.ActivationFunctionType.Exp
    MULT = mybir.AluOpType.mult
    ADD = mybir.AluOpType.add

    with ExitStack() as ctx:
        pp = ctx.enter_context(tc.tile_pool(name="persist", bufs=1))

        # ---- persistent SBUF ----
        xhl = pp.tile([128, KC, 2, S], F8, name="xhl", tag="xhl")
        wq8 = pp.tile([128, KC, HV], F8, name="wq8", tag="wq8")
        wk8 = pp.tile([128, KC, HV], F8, name="wk8", tag="wk8")
        wv8 = [pp.tile([128, KC, HV], F8, name=f"wv8{i}", tag=f"wv8{i}") for i in range(2)]
        wo_sb = pp.tile([128, NPAIR, DM], BF16, name="wo_sb", tag="wo_sb")
        # Q^T/K^T per pair: [128, 2, S] fp8; block 1 stays zero (DoubleRow pad)
        qt8 = [pp.tile([128, 2, S], F8, name=f"qt8{p}", tag=f"qt8{p}") for p in range(NPAIR)]
        kt8 = [pp.tile([128, 2, S], F8, name=f"kt8{p}", tag=f"kt8{p}") for p in range(NPAIR)]
        # V' bf16: [128 kpos, s-chunk, head, 64+ones]
        vpr = pp.tile([128, S // 128, 8, 65], BF16, name="vpr", tag="vpr")
        at_sb = [pp.tile([128, S], BF16, name=f"at{p}", tag=f"at{p}") for p in range(NPAIR)]
        mask_sb = pp.tile([128, 128], BF16, name="mask_sb", tag="mask_sb")
        bq_sb = pp.tile([128, NPAIR], F32, name="bq_sb", tag="bq_sb")
        bk_sb = pp.tile([128, NPAIR], F32, name="bk_sb", tag="bk_sb")
        ones_sb = pp.tile([1, 64], mybir.dt.float32r, name="ones_sb", tag="ones_sb")

        psum = ctx.enter_context(tc.tile_pool(name="psum", bufs=2, space="PSUM"))
        # Banks: st 2x2 + au 2x1 + pj 2x1 = 8.

        # ---- zero pads / ones columns (before any use) ----
        for p in range(NPAIR):
            nc.gpsimd.memset(qt8[p][:, 1, :].bitcast(F32), 0.0)
            nc.gpsimd.memset(kt8[p][:, 1, :].bitcast(F32), 0.0)
        nc.gpsimd.memset(vpr[:, :, :, 64:65], 1.0)
        nc.gpsimd.memset(ones_sb[:].bitcast(F32), 1.0)

        # ---- DMA issue order: tiny first, then what phase A consumes first ----
        # First Q/K tile (q-cols 0:512) gates the exp pipeline: weights first
        # (so the PE chases the x chunks as they land), then its x columns,
        # then the rest streams in 512-col pieces.  x hi/lo are packed in one
        # dram tensor so each piece is one DMA.
        nc.sync.dma_start(out=wq8[:], in_=wqh.rearrange("(c p) m -> p c m", p=128))
        nc.sync.dma_start(out=wk8[:], in_=wkh.rearrange("(c p) m -> p c m", p=128))
        nc.sync.dma_start(out=bq_sb[:], in_=bq.rearrange("(pair r) -> r pair", r=128))
        nc.sync.dma_start(out=bk_sb[:], in_=bk.rearrange("(pair r) -> r pair", r=128))
        for kc in range(KC):
            nc.sync.dma_start(
                out=xhl[:, kc, :, 0:512], in_=x8[kc * 128:(kc + 1) * 128, :, 0:512]
            )
        nc.sync.dma_start(out=mask_sb[:], in_=masks)
        nc.sync.dma_start(out=wv8[0][:], in_=wvh.rearrange("(c p) m -> p c m", p=128))
        nc.sync.dma_start(out=wv8[1][:], in_=wvl.rearrange("(c p) m -> p c m", p=128))
        for s0 in range(512, S, 512):
            for kc in range(KC):
                nc.sync.dma_start(
                    out=xhl[:, kc, :, s0:s0 + 512],
                    in_=x8[kc * 128:(kc + 1) * 128, :, s0:s0 + 512],
                )
        nc.sync.dma_start(
            out=wo_sb[:],
            in_=wo.rearrange("(pair p) c -> p pair c", p=128),
        )

        pt_pool = ctx.enter_context(tc.tile_pool(name="pt", bufs=18))
        pr_pool = ctx.enter_context(tc.tile_pool(name="pr", bufs=4))
        rbs_pool = ctx.enter_context(tc.tile_pool(name="rbs", bufs=4))
        osb_pool = ctx.enter_context(tc.tile_pool(name="osb", bufs=3))

        F32R = mybir.dt.float32r

        def qk_half(p, nt, which, terms=1):
            """Project Q^T or K^T for pair p, q-tile nt -> fp8 eviction.

            1-term (x_hi @ W_hi): the fp8 eviction noise dominates the
            dropped correction terms.
            """
            qs = nt * 512
            last = terms * (KC // 2) - 1
            w8, t8, b_sb = ((wq8, qt8, bq_sb), (wk8, kt8, bk_sb))[which]
            ps = psum.tile([128, 512], F32, name=f"qk_{which}_{p}_{nt}", tag="pj")
            n = 0
            for xi in range(terms):
                for pc in range(KC // 2):
                    nc.tensor.matmul(
                        ps[:],
                        lhsT=w8[:, 2 * pc:2 * pc + 2, p * 128:(p + 1) * 128],
                        rhs=xhl[:, 2 * pc:2 * pc + 2, xi, qs:qs + 512],
                        start=(n == 0),
                        stop=(n == last),
                        perf_mode=mybir.MatmulPerfMode.DoubleRow,
                    )
                    n += 1
            with nc.allow_low_precision(reason="fp8 eviction is the design"):
                nc.vector.tensor_scalar(
                    out=t8[p][:, 0, qs:qs + 512],
                    in0=ps[:],
                    scalar1=EVS,
                    scalar2=b_sb[:, p:p + 1],
                    op0=MULT,
                    op1=ADD,
                )

        def qk_tile(p, nt, terms=1):
            qk_half(p, nt, 0, terms)
            qk_half(p, nt, 1, terms)

        def v_fillers(sc):
            """Project V for s-chunk sc -> bf16 V' with ones column.

            Returned as two issue-thunks so the 12-matmul group can be
            spread across scores/exp slots.
            """
            ref = {}
            terms = ((0, 0), (1, 0), (0, 1))

            def part(lo, hi, first, last):
                if first:
                    ref["ps"] = psum.tile([128, 512], F32, name=f"v_{sc}", tag="pj")
                ps = ref["ps"]
                for n in range(lo, hi):
                    xi, wi = terms[n // (KC // 2)]
                    pc = n % (KC // 2)
                    nc.tensor.matmul(
                        ps[:],
                        lhsT=xhl[:, 2 * pc:2 * pc + 2, xi, sc * 128:(sc + 1) * 128],
                        rhs=wv8[wi][:, 2 * pc:2 * pc + 2, :],
                        start=(n == 0),
                        stop=(n == 11),
                        perf_mode=mybir.MatmulPerfMode.DoubleRow,
                    )
                if last:
                    with nc.allow_low_precision(reason="bf16 V"):
                        nc.vector.tensor_scalar_mul(
                            out=vpr[:, sc, :, 0:64],
                            in0=ps[:].rearrange("p (h c) -> p h c", h=8),
                            scalar1=EVS,
                        )

            return [
                lambda: part(0, 6, True, False),
                lambda: part(6, 12, False, True),
            ]

        def st_exp_pc(h, j, pc):
            """One S^T pair-tile + exp (+ diag masks) -> pt [128, 2, 512]."""
            p, hp = divmod(h, 2)
            r0 = hp * 64
            vp = max(0, 128 * (2 * pc) - 512 * j)
            st = psum.tile([128, 1024], F32, name=f"st_{h}_{j}_{pc}", tag="st")
            st3 = st[:].rearrange("p (c q) -> p c q", c=2)
            for c in range(2):
                kc = 2 * pc + c
                nc.tensor.matmul(
                    st3[:, c, vp:512],
                    lhsT=kt8[p][r0:r0 + 64, :, kc * 128:(kc + 1) * 128],
                    rhs=qt8[p][r0:r0 + 64, :, j * 512 + vp:(j + 1) * 512],
                    start=True,
                    stop=True,
                    perf_mode=mybir.MatmulPerfMode.DoubleRow,
                )
            pt = pt_pool.tile([128, 2, 512], BF16, name=f"pt_{h}_{j}_{pc}", tag="pt")
            nc.scalar.activation(pt[:, :, vp:512], st3[:, :, vp:512], EXP, scale=EXPS)
            for c in range(2):
                kc = 2 * pc + c
                i = kc - 4 * j
                if i >= 0:  # diagonal chunk: triangular 0/1 mask
                    q0 = 128 * i
                    nc.vector.tensor_tensor(
                        out=pt[:, c, q0:q0 + 128],
                        in0=pt[:, c, q0:q0 + 128],
                        in1=mask_sb[:],
                        op=MULT,
                    )
            return pt

        class AvState:
            """v-major AV accumulation au^T[65, 512] for one (h, j), issued
            one k-chunk at a time so it can interleave with later heads'
            scores (fills the PE bubbles while ACT drains st tiles).  The
            PSUM tile is allocated lazily at the first step so queued states
            don't hold banks."""

            def __init__(self, h, j, pts):
                self.h, self.j, self.pts = h, j, pts
                self.nk = 4 * j + 4
                self.kc = 0
                self.au = None

            def can_step(self):
                return self.kc < self.nk and self.kc // 2 < len(self.pts)

            def step(self):
                if not self.can_step():
                    return self.kc < self.nk
                if self.au is None:
                    self.au = psum.tile(
                        [65, 512], F32, name=f"au_{self.h}_{self.j}", tag="au"
                    )
                kc = self.kc
                vc = max(0, 128 * kc - 512 * self.j)
                nc.tensor.matmul(
                    self.au[:, vc:512],
                    lhsT=vpr[:, kc, self.h, :],
                    rhs=self.pts[kc // 2][:, kc % 2, vc:512],
                    start=(kc == 0),
                    stop=(kc == self.nk - 1),
                )
                self.kc += 1
                return self.kc < self.nk

            def finish_recip(self):
                while self.kc < self.nk:
                    self.step()
                self.r_sb = pr_pool.tile(
                    [1, 512], F32R, name=f"r_{self.h}_{self.j}", tag="r"
                )
                with nc.allow_low_precision(reason="f32r out is bit-identical"):
                    nc.vector.reciprocal(out=self.r_sb[:], in_=self.au[64:65, :])
                self.rb_sb = rbs_pool.tile(
                    [64, 512], F32R, name=f"rbs_{self.h}_{self.j}", tag="rbs"
                )
                nc.gpsimd.partition_broadcast(self.rb_sb[:], self.r_sb[:])

            def finish_rb(self):
                h, j, au = self.h, self.j, self.au
                p, hp = divmod(h, 2)
                r0 = hp * 64
                with nc.allow_low_precision(reason="bf16 attn out"):
                    nc.vector.tensor_tensor(
                        out=at_sb[p][r0:r0 + 64, j * 512:(j + 1) * 512],
                        in0=au[0:64, :],
                        in1=self.rb_sb[:],
                        op=MULT,
                    )

        def out_fillers(sc):
            """Output projection for one 128-row s-chunk, as two issue-thunks."""
            ref = {}

            def m_part(m):
                if m == 0:
                    ref["osb"] = osb_pool.tile(
                        [128, DM], BF16, name=f"osb_{sc}", tag="osb"
                    )
                osb = ref["osb"]
                op_ps = psum.tile([128, 512], F32, name=f"o_{sc}_{m}", tag="pj")
                for p in range(NPAIR):
                    nc.tensor.matmul(
                        op_ps[:],
                        lhsT=at_sb[p][:, sc * 128:(sc + 1) * 128],
                        rhs=wo_sb[:, p, m * 512:(m + 1) * 512],
                        start=(p == 0),
                        stop=(p == NPAIR - 1),
                    )
                with nc.allow_low_precision(reason="bf16 out"):
                    nc.vector.tensor_copy(
                        out=osb[:, m * 512:(m + 1) * 512], in_=op_ps[:]
                    )
                if m == 1:
                    nc.sync.dma_start(out=o[sc * 128:(sc + 1) * 128, :], in_=osb[:])

            return [lambda: m_part(0), lambda: m_part(1)]

        def out_sc(sc):
            for f in out_fillers(sc):
                f()

        # ---- interleaved issue schedule ----
        # Only pair 0's first Q/K tile is issued up front (it gates the first
        # exp); everything else — remaining Q/K tiles, V tiles, output
        # chunks — is spread as fillers through the scores/exp stream,
        # placed after their DMA arrivals and before their AV deadlines.
        # AV accumulations run through a queue pumped in the PE bubbles.
        av_q = []     # AvStates pending steps, oldest first
        ripe = []     # stepped out; recip+broadcast issued, await finish_rb

        def pump(n):
            while n > 0 and av_q:
                s = av_q[0]
                if not s.can_step():
                    break
                alive = s.step()
                n -= 1
                if not alive:
                    s.finish_recip()
                    if ripe:
                        ripe.pop(0).finish_rb()
                    ripe.append(s)
                    av_q.pop(0)

        def vf(sc):
            """Whole V tile as one filler: the 12-matmul PSUM group must stay
            closed before any other pj-tag allocation (open groups in a
            reused slot would be clobbered)."""
            parts = v_fillers(sc)

            def run():
                for f in parts:
                    f()

            return run

        def qf(p, nt):
            return [lambda: qk_half(p, nt, 0), lambda: qk_half(p, nt, 1)]

        # filler lists per (j, h); deadlines: qt/kt tile nt before (j=nt, h=2p);
        # v(sc) before the AV chunk kc=sc is pumped; v tiles no earlier than
        # the wv DMA arrivals (~15us).
        FILL = {
            (0, 0): qf(1, 0),
            (0, 1): qf(2, 0),
            (0, 2): qf(3, 0) + [vf(0), vf(1)],
            (0, 3): [vf(2), vf(3)],
            (0, 4): [vf(4), vf(5)],
            (0, 5): [vf(6)],
            (0, 6): qf(0, 1) + [vf(7)],
            (0, 7): qf(1, 1),
            (1, 0): qf(2, 1) + [vf(8)],
            (1, 1): qf(3, 1) + [vf(9)],
            (1, 2): qf(0, 2) + out_fillers(0),
            (1, 3): qf(1, 2) + out_fillers(1),
            (1, 4): qf(2, 2) + out_fillers(2),
            (1, 5): qf(3, 2) + out_fillers(3),
            (1, 6): [vf(10)],
            (1, 7): [vf(11)],
            (2, 0): [vf(12)],
            (2, 1): [vf(13)],
            (2, 2): qf(0, 3) + out_fillers(4),
            (2, 3): qf(1, 3) + out_fillers(5),
            (2, 4): qf(2, 3) + out_fillers(6),
            (2, 5): qf(3, 3) + out_fillers(7),
            (2, 6): [vf(14)],
            (2, 7): [vf(15)],
            (3, 2): out_fillers(8),
            (3, 3): out_fillers(9),
            (3, 4): out_fillers(10),
            (3, 5): out_fillers(11),
        }

        qk_tile(0, 0)
        for j in range(NQT):
            for h in range(8):
                fillers = list(FILL.get((j, h), []))
                rate = 3 if (j == 0 and h >= 3) or (j, h) in ((1, 0), (3, 7)) else 2
                if j == 0 and h < 3:
                    rate = 0
                pts = []
                av_q.append(AvState(h, j, pts))
                for pc in range(2 * j + 2):
                    pts.append(st_exp_pc(h, j, pc))
                    if fillers:
                        fillers.pop(0)()
                    pump(rate)
                for f in fillers:
                    f()
        while av_q:
            pump(4)
        while ripe:
            ripe.pop(0).finish_rb()
        for sc in range(12, 16):
            out_sc(sc)


def _masks_np():
    # tri[r, c] = 1 where k_local <= q_local (unmasked on the diagonal block)
    r = np.arange(128)[:, None]
    c = np.arange(128)[None, :]
    return (c >= r).astype(ml_dtypes.bfloat16)


def _hilo(a, s):
    """Same-scale fp8 hi/lo split: a ~ (hi + lo)/s with hi, lo fp8e4."""
    hi = np.asarray(a * s, ml_dtypes.float8_e4m3)
    lo = np.asarray((a * s - hi.astype(np.float32)), ml_dtypes.float8_e4m3)
    return np.ascontiguousarray(hi), np.ascontiguousarray(lo)


def make_in_maps(input, Wq, bq, Wk, bk, Wv, Wo):
    scale = np.float32(1.0 / np.sqrt(D_K))
    masks = _masks_np()
    input = np.asarray(input, np.float32)
    Wq = np.asarray(Wq, np.float32)
    Wk = np.asarray(Wk, np.float32)
    Wv = np.asarray(Wv, np.float32)
    Wo = np.asarray(Wo, np.float32)
    bq = np.asarray(bq, np.float32)
    bk = np.asarray(bk, np.float32)
    in_maps = []
    xsplit = []
    for b in range(BATCH):
        xhi, xlo = _hilo(input[b].T, SX)
        xsplit.append(np.ascontiguousarray(np.stack([xhi, xlo], axis=1)))
    for c in range(NCORES):
        b, g = divmod(c, 2)
        cols = slice(g * HV, (g + 1) * HV)
        wqh, _ = _hilo(Wq[:, cols] * (scale * SQ), SW)
        wkh, _ = _hilo(Wk[:, cols] * SQ, SW)
        wvh, wvl = _hilo(Wv[:, cols], SW)
        in_maps.append(
            {
                "x8": xsplit[b],
                "wqh": wqh,
                "wkh": wkh,
                "wvh": wvh, "wvl": wvl,
                "wo": np.ascontiguousarray(
                    np.asarray(Wo[g * HV:(g + 1) * HV, :], ml_dtypes.bfloat16)
                ),
                "bq": np.ascontiguousarray(bq[cols] * (scale * SQ)),
                "bk": np.ascontiguousarray(bk[cols] * SQ),
                "masks": masks,
            }
        )
    return in_maps


def _numpy_fallback(input, attn_mask, Wq, bq, Wk, bk, Wv, bv, Wo, bo):
    """Host fallback for non-causal masks (should not trigger in practice)."""
    x = np.asarray(input, np.float32)
    mask = np.asarray(attn_mask)
    B, S_, _ = x.shape
    scale = np.float32(1.0 / np.sqrt(D_K))
    out = np.empty((B, S_, D_MODEL), np.float32)
    for b in range(B):
        q = (x[b] @ Wq + bq).reshape(S_, N_HEAD, D_K)
        k = (x[b] @ Wk + bk).reshape(S_, N_HEAD, D_K)
        v = (x[b] @ Wv + bv).reshape(S_, N_HEAD, D_V)
        attn = np.empty((S_, N_HEAD, D_V), np.float32)
        for h in range(N_HEAD):
            score = (q[:, h] @ k[:, h].T) * scale
            score = np.where(mask, -np.inf, score)
            score -= score.max(axis=-1, keepdims=True)
            p = np.exp(score)
            p /= p.sum(axis=-1, keepdims=True)
            attn[:, h] = p @ v[:, h]
        out[b] = attn.reshape(S_, N_HEAD * D_V) @ Wo + bo
    return out


_CACHED_RUNNER = None


def _make_runner(nc):
    """Build the shard_map-jitted PJRT executor once; reuse across calls."""
    import jax
    from jax.sharding import Mesh, PartitionSpec
    from jax.experimental.shard_map import shard_map
    from concourse import bass2jax

    bass2jax.install_neuronx_cc_hook()
    partition_name = nc.partition_id_tensor.name if nc.partition_id_tensor else None
    in_names, out_names, out_avals, zero_outs = [], [], [], []
    for alloc in nc.m.functions[0].allocations:
        if not isinstance(alloc, mybir.MemoryLocationSet):
            continue
        name = alloc.memorylocations[0].name
        if alloc.kind == "ExternalInput":
            if name != partition_name:
                in_names.append(name)
        elif alloc.kind == "ExternalOutput":
            out_names.append(name)
            shape = tuple(alloc.tensor_shape)
            dtype = mybir.dt.np(alloc.dtype)
            out_avals.append(jax.core.ShapedArray(shape, dtype))
            zero_outs.append(np.zeros(shape, dtype))
    n_params = len(in_names)
    n_outs = len(out_avals)
    all_in_names = list(in_names) + list(out_names)
    if partition_name is not None:
        all_in_names.append(partition_name)

    def _body(*args):
        operands = list(args)
        if partition_name is not None:
            operands.append(bass2jax.partition_id_tensor())
        outs = bass2jax._bass_exec_p.bind(
            *operands,
            out_avals=tuple(out_avals),
            in_names=tuple(all_in_names),
            out_names=tuple(out_names),
            lowering_input_output_aliases=(),
            sim_require_finite=True,
            sim_require_nnan=True,
            nc=nc,
        )
        return tuple(outs)

    devices = jax.devices()[:NCORES]
    mesh = Mesh(np.asarray(devices), ("core",))
    sharded = jax.jit(
        shard_map(
            _body,
            mesh=mesh,
            in_specs=(PartitionSpec("core"),) * (n_params + n_outs),
            out_specs=(PartitionSpec("core"),) * n_outs,
            check_rep=False,
        ),
        donate_argnums=tuple(range(n_params, n_params + n_outs)),
        keep_unused=True,
    )

    def run(in_maps):
        concat_in = [
            np.concatenate(
                [np.asarray(in_maps[c][nm]) for c in range(NCORES)], axis=0
            )
            for nm in in_names
        ]
        concat_zeros = [
            np.zeros((NCORES * z.shape[0], *z.shape[1:]), z.dtype) for z in zero_outs
        ]
        out_arrs = sharded(*concat_in, *concat_zeros)
        return [
            {
                nm: np.asarray(out_arrs[i]).reshape(NCORES, *out_avals[i].shape)[c]
                for i, nm in enumerate(out_names)
            }
            for c in range(NCORES)
        ]

    return run


def kernel(input, attn_mask, Wq, bq, Wk, bk, Wv, bv, Wo, bo):
    causal = np.triu(np.ones((SEQ, SEQ), bool), k=1)
    if not np.array_equal(np.asarray(attn_mask), causal):
        return _numpy_fallback(input, attn_mask, Wq, bq, Wk, bk, Wv, bv, Wo, bo)

    global _CACHED_NC, _CACHED_RUNNER
    if _CACHED_NC is None:
        _CACHED_NC = _build_nc()

    in_maps = make_in_maps(input, Wq, bq, Wk, bk, Wv, Wo)
    try:
        if _CACHED_RUNNER is None:
            _CACHED_RUNNER = _make_runner(_CACHED_NC)
        outs = _CACHED_RUNNER(in_maps)
    except Exception:
        # jit-caching fast path failed (e.g. jax version skew) — use the
        # stock executor.
        _CACHED_RUNNER = None
        outs = bass_utils.run_bass_kernel_spmd(
            _CACHED_NC, in_maps, core_ids=list(range(NCORES))
        ).results

    corr = (
        np.asarray(bv, np.float32) @ np.asarray(Wo, np.float32)
        + np.asarray(bo, np.float32)
    ).astype(np.float32)
    out = np.empty((BATCH, SEQ, D_MODEL), np.float32)
    for b in range(BATCH):
        out[b] = (
            outs[2 * b]["o"].astype(np.float32)
            + outs[2 * b + 1]["o"].astype(np.float32)
            + corr[None, :]
        )
    return out


# revision 64
# speedup vs baseline: 1.0451x; 1.0451x over previous
"""Multi-head causal attention (B=4, S=2048, D=1024, H=16, dk=dv=64) on 8 NeuronCores.

Sharding: core c -> (batch b = c//2, head-group g = c%2 of 8 heads).
Each core computes Q/K/V projections for its batch restricted to its 8 heads,
causal softmax attention, and a partial output projection with its 512 rows of
Wo.  The host sums the two partials per batch and adds the constant correction
bv @ Wo + bo (bv passes through attention linearly because softmax rows sum
to 1).

Dtype strategy (keyed to the TRN2 matmul cost model: cost = out_free x
cycles_per_row; fp8 DoubleRow = 0.5 c/r with 2x128 contraction per
instruction, everything else 1.0):
  - QKV projections: x and W shipped from host as same-scale fp8e4 hi/lo
    pairs; 3-term (hi.hi + lo.hi + hi.lo) DoubleRow accumulation -> 0.75
    cycles per 128-contraction chunk instead of 1.0, with ~bf16 accuracy.
  - Scores: Q^T/K^T evicted to fp8e4 (x2 / x16 scales folded host-side)
    stored as [*, 2, S] with a zero second block, so a single DoubleRow
    matmul with d_k=64 contraction costs 0.5 c/r (the zero block contributes
    nothing and is free).
  - exp on ACT with scale=1/256 folding the score descale; bf16 P out.
  - AV v-major: au^T[65, 512] per (h, q-tile) with the ones row of V' as
    softmax denominator; bf16 rhs = exp output.  Big 512-col matmuls keep
    the PE sequencer (131 ns per Ldweights+Matmult pair in the cost model)
    off the critical path.
  - Normalization: reciprocal_approx_fast on the denominator row, broadcast
    across 64 partitions with a tiny f32r matmul, one tensor_tensor multiply
    into bf16 A^T.
  - Output projection and DMA in bf16.

Engine budget per core: ACT (exp, ~143us) is the bottleneck; PE ~155us of
issue interleaved so scores start ~10us in; DVE/Pool carry evictions,
reciprocals, masks and copies.
"""

import numpy as np
import ml_dtypes
from contextlib import ExitStack

import concourse.bass as bass
import concourse.mybir as mybir
import concourse.tile as tile
from concourse import bacc, bass_utils

N_HEAD, D_MODEL, D_K, D_V = 16, 1024, 64, 64
BATCH, SEQ = 4, 2048
NCORES = 8
S = SEQ
DM = D_MODEL
HV = 8 * D_V          # 512 local head-value columns per core
KC = DM // 128        # 8 d_model chunks
NPAIR = 4             # local head pairs
NQT = S // 512        # 4 q-tiles
F32 = mybir.dt.float32
BF16 = mybir.dt.bfloat16
F8 = mybir.dt.float8e4

SX = 16.0             # fp8 scale for x (hi and lo use the same scale)
SW = 8.0              # fp8 scale for projection weights
SQ = 16.0             # extra scale on the Q/K paths so fp8 eviction is exact
EVS = 1.0 / (SX * SW)          # psum -> Q/K/V descale
EXPS = 1.0 / (2.0 * SQ * 8.0)  # q8*k8 -> exp argument (incl. 1/sqrt(dk))

_CACHED_NC = None


def _build_nc(nbody=1, phases="ABC"):
    nc = bacc.Bacc("TRN2", target_bir_lowering=False, debug=False)

    x8 = nc.dram_tensor("x8", [DM, 2, S], F8, kind="ExternalInput").ap()
    wqh = nc.dram_tensor("wqh", [DM, HV], F8, kind="ExternalInput").ap()
    wkh = nc.dram_tensor("wkh", [DM, HV], F8, kind="ExternalInput").ap()
    wvh = nc.dram_tensor("wvh", [DM, HV], F8, kind="ExternalInput").ap()
    wvl = nc.dram_tensor("wvl", [DM, HV], F8, kind="ExternalInput").ap()
    wo = nc.dram_tensor("wo", [HV, DM], BF16, kind="ExternalInput").ap()
    bq = nc.dram_tensor("bq", [HV], F32, kind="ExternalInput").ap()
    bk = nc.dram_tensor("bk", [HV], F32, kind="ExternalInput").ap()
    masks = nc.dram_tensor("masks", [128, 128], BF16, kind="ExternalInput").ap()
    o = nc.dram_tensor("o", [S, DM], BF16, kind="ExternalOutput").ap()

    with tile.TileContext(nc) as tc:
        for _ in range(nbody):
            _build_kernel(tc, nc, x8, wqh, wkh, wvh, wvl,
                          wo, bq, bk, masks, o)
    nc.compile()
    return nc


def _build_kernel(tc, nc, x8, wqh, wkh, wvh, wvl,
                  wo, bq, bk, masks, o):
    EXP = mybir.ActivationFunctionType.Exp
    MULT = mybir.AluOpType.mult
    ADD = mybir.AluOpType.add

    with ExitStack() as ctx:
        pp = ctx.enter_context(tc.tile_pool(name="persist", bufs=1))

        # ---- persistent SBUF ----
        xhl = pp.tile([128, KC, 2, S], F8, name="xhl", tag="xhl")
        wq8 = pp.tile([128, KC, HV], F8, name="wq8", tag="wq8")
        wk8 = pp.tile([128, KC, HV], F8, name="wk8", tag="wk8")
        wv8 = [pp.tile([128, KC, HV], F8, name=f"wv8{i}", tag=f"wv8{i}") for i in range(2)]
        wo_sb = pp.tile([128, NPAIR, DM], BF16, name="wo_sb", tag="wo_sb")
        # Q^T/K^T per pair: [128, 2, S] fp8; block 1 stays zero (DoubleRow pad)
        qt8 = [pp.tile([128, 2, S], F8, name=f"qt8{p}", tag=f"qt8{p}") for p in range(NPAIR)]
        kt8 = [pp.tile([128, 2, S], F8, name=f"kt8{p}", tag=f"kt8{p}") for p in range(NPAIR)]
        # V' bf16: [128 kpos, s-chunk, head, 64+ones]
        vpr = pp.tile([128, S // 128, 8, 65], BF16, name="vpr", tag="vpr")
        at_sb = [pp.tile([128, S], BF16, name=f"at{p}", tag=f"at{p}") for p in range(NPAIR)]
        mask_sb = pp.tile([128, 128], BF16, name="mask_sb", tag="mask_sb")
        bq_sb = pp.tile([128, NPAIR], F32, name="bq_sb", tag="bq_sb")
        bk_sb = pp.tile([128, NPAIR], F32, name="bk_sb", tag="bk_sb")
        ones_sb = pp.tile([1, 64], mybir.dt.float32r, name="ones_sb", tag="ones_sb")

        psum = ctx.enter_context(tc.tile_pool(name="psum", bufs=2, space="PSUM"))
        # Banks: st 2x2 + au 2x1 + pj 2x1 = 8.

        # ---- zero pads / ones columns (before any use) ----
        for p in range(NPAIR):
            nc.gpsimd.memset(qt8[p][:, 1, :].bitcast(F32), 0.0)
            nc.gpsimd.memset(kt8[p][:, 1, :].bitcast(F32), 0.0)
        nc.gpsimd.memset(vpr[:, :, :, 64:65], 1.0)
        nc.gpsimd.memset(ones_sb[:].bitcast(F32), 1.0)

        # ---- DMA issue order: tiny first, then what phase A consumes first ----
        # First Q/K tile (q-cols 0:512) gates the exp pipeline: weights first
        # (so the PE chases the x chunks as they land), then its x columns,
        # then the rest streams in 512-col pieces.  x hi/lo are packed in one
        # dram tensor so each piece is one DMA.
        nc.sync.dma_start(out=wq8[:], in_=wqh.rearrange("(c p) m -> p c m", p=128))
        nc.sync.dma_start(out=wk8[:], in_=wkh.rearrange("(c p) m -> p c m", p=128))
        nc.sync.dma_start(out=bq_sb[:], in_=bq.rearrange("(pair r) -> r pair", r=128))
        nc.sync.dma_start(out=bk_sb[:], in_=bk.rearrange("(pair r) -> r pair", r=128))
        for kc in range(KC):
            nc.sync.dma_start(
                out=xhl[:, kc, :, 0:512], in_=x8[kc * 128:(kc + 1) * 128, :, 0:512]
            )
        nc.sync.dma_start(out=mask_sb[:], in_=masks)
        nc.sync.dma_start(out=wv8[0][:], in_=wvh.rearrange("(c p) m -> p c m", p=128))
        nc.sync.dma_start(out=wv8[1][:], in_=wvl.rearrange("(c p) m -> p c m", p=128))
        for s0 in range(512, S, 512):
            for kc in range(KC):
                nc.sync.dma_start(
                    out=xhl[:, kc, :, s0:s0 + 512],
                    in_=x8[kc * 128:(kc + 1) * 128, :, s0:s0 + 512],
                )
        nc.sync.dma_start(
            out=wo_sb[:],
            in_=wo.rearrange("(pair p) c -> p pair c", p=128),
        )

        pt_pool = ctx.enter_context(tc.tile_pool(name="pt", bufs=18))
        pr_pool = ctx.enter_context(tc.tile_pool(name="pr", bufs=4))
        rbs_pool = ctx.enter_context(tc.tile_pool(name="rbs", bufs=4))
        osb_pool = ctx.enter_context(tc.tile_pool(name="osb", bufs=3))

        F32R = mybir.dt.float32r

        def qk_half(p, nt, which, terms=1):
            """Project Q^T or K^T for pair p, q-tile nt -> fp8 eviction.

            1-term (x_hi @ W_hi): the fp8 eviction noise dominates the
            dropped correction terms.
            """
            qs = nt * 512
            last = terms * (KC // 2) - 1
            w8, t8, b_sb = ((wq8, qt8, bq_sb), (wk8, kt8, bk_sb))[which]
            ps = psum.tile([128, 512], F32, name=f"qk_{which}_{p}_{nt}", tag="pj")
            n = 0
            for xi in range(terms):
                for pc in range(KC // 2):
                    nc.tensor.matmul(
                        ps[:],
                        lhsT=w8[:, 2 * pc:2 * pc + 2, p * 128:(p + 1) * 128],
                        rhs=xhl[:, 2 * pc:2 * pc + 2, xi, qs:qs + 512],
                        start=(n == 0),
                        stop=(n == last),
                        perf_mode=mybir.MatmulPerfMode.DoubleRow,
                    )
                    n += 1
            with nc.allow_low_precision(reason="fp8 eviction is the design"):
                nc.vector.tensor_scalar(
                    out=t8[p][:, 0, qs:qs + 512],
                    in0=ps[:],
                    scalar1=EVS,
                    scalar2=b_sb[:, p:p + 1],
                    op0=MULT,
                    op1=ADD,
                )

        def qk_tile(p, nt, terms=1):
            qk_half(p, nt, 0, terms)
            qk_half(p, nt, 1, terms)

        def v_fillers(sc):
            """Project V for s-chunk sc -> bf16 V' with ones column.

            Returned as two issue-thunks so the 12-matmul group can be
            spread across scores/exp slots.
            """
            ref = {}
            terms = ((0, 0), (1, 0), (0, 1))

            def part(lo, hi, first, last):
                if first:
                    ref["ps"] = psum.tile([128, 512], F32, name=f"v_{sc}", tag="pj")
                ps = ref["ps"]
                for n in range(lo, hi):
                    xi, wi = terms[n // (KC // 2)]
                    pc = n % (KC // 2)
                    nc.tensor.matmul(
                        ps[:],
                        lhsT=xhl[:, 2 * pc:2 * pc + 2, xi, sc * 128:(sc + 1) * 128],
                        rhs=wv8[wi][:, 2 * pc:2 * pc + 2, :],
                        start=(n == 0),
                        stop=(n == 11),
                        perf_mode=mybir.MatmulPerfMode.DoubleRow,
                    )
                if last:
                    with nc.allow_low_precision(reason="bf16 V"):
                        nc.vector.tensor_scalar_mul(
                            out=vpr[:, sc, :, 0:64],
                            in0=ps[:].rearrange("p (h c) -> p h c", h=8),
                            scalar1=EVS,
                        )

            return [
                lambda: part(0, 6, True, False),
                lambda: part(6, 12, False, True),
            ]

        def st_exp_pc(h, j, pc):
            """One S^T pair-tile + exp (+ diag masks) -> pt [128, 2, 512]."""
            p, hp = divmod(h, 2)
            r0 = hp * 64
            vp = max(0, 128 * (2 * pc) - 512 * j)
            st = psum.tile([128, 1024], F32, name=f"st_{h}_{j}_{pc}", tag="st")
            st3 = st[:].rearrange("p (c q) -> p c q", c=2)
            for c in range(2):
                kc = 2 * pc + c
                nc.tensor.matmul(
                    st3[:, c, vp:512],
                    lhsT=kt8[p][r0:r0 + 64, :, kc * 128:(kc + 1) * 128],
                    rhs=qt8[p][r0:r0 + 64, :, j * 512 + vp:(j + 1) * 512],
                    start=True,
                    stop=True,
                    perf_mode=mybir.MatmulPerfMode.DoubleRow,
                )
            pt = pt_pool.tile([128, 2, 512], BF16, name=f"pt_{h}_{j}_{pc}", tag="pt")
            nc.scalar.activation(pt[:, :, vp:512], st3[:, :, vp:512], EXP, scale=EXPS)
            for c in range(2):
                kc = 2 * pc + c
                i = kc - 4 * j
                if i >= 0:  # diagonal chunk: triangular 0/1 mask
                    q0 = 128 * i
                    nc.vector.tensor_tensor(
                        out=pt[:, c, q0:q0 + 128],
                        in0=pt[:, c, q0:q0 + 128],
                        in1=mask_sb[:],
                        op=MULT,
                    )
            return pt

        class AvState:
            """v-major AV accumulation au^T[65, 512] for one (h, j), issued
            one k-chunk at a time so it can interleave with later heads'
            scores (fills the PE bubbles while ACT drains st tiles).  The
            PSUM tile is allocated lazily at the first step so queued states
            don't hold banks."""

            def __init__(self, h, j, pts):
                self.h, self.j, self.pts = h, j, pts
                self.nk = 4 * j + 4
                self.kc = 0
                self.au = None

            def can_step(self):
                return self.kc < self.nk and self.kc // 2 < len(self.pts)

            def step(self):
                if not self.can_step():
                    return self.kc < self.nk
                if self.au is None:
                    self.au = psum.tile(
                        [65, 512], F32, name=f"au_{self.h}_{self.j}", tag="au"
                    )
                kc = self.kc
                vc = max(0, 128 * kc - 512 * self.j)
                nc.tensor.matmul(
                    self.au[:, vc:512],
                    lhsT=vpr[:, kc, self.h, :],
                    rhs=self.pts[kc // 2][:, kc % 2, vc:512],
                    start=(kc == 0),
                    stop=(kc == self.nk - 1),
                )
                self.kc += 1
                return self.kc < self.nk

            def finish_recip(self):
                while self.kc < self.nk:
                    self.step()
                self.r_sb = pr_pool.tile(
                    [1, 512], F32R, name=f"r_{self.h}_{self.j}", tag="r"
                )
                with nc.allow_low_precision(reason="f32r out is bit-identical"):
                    nc.vector.reciprocal(out=self.r_sb[:], in_=self.au[64:65, :])
                self.rb_sb = rbs_pool.tile(
                    [64, 512], F32R, name=f"rbs_{self.h}_{self.j}", tag="rbs"
                )
                nc.gpsimd.partition_broadcast(self.rb_sb[:], self.r_sb[:])

            def finish_rb(self):
                h, j, au = self.h, self.j, self.au
                p, hp = divmod(h, 2)
                r0 = hp * 64
                with nc.allow_low_precision(reason="bf16 attn out"):
                    nc.vector.tensor_tensor(
                        out=at_sb[p][r0:r0 + 64, j * 512:(j + 1) * 512],
                        in0=au[0:64, :],
                        in1=self.rb_sb[:],
                        op=MULT,
                    )

        def out_fillers(sc):
            """Output projection for one 128-row s-chunk, as two issue-thunks."""
            ref = {}

            def m_part(m):
                if m == 0:
                    ref["osb"] = osb_pool.tile(
                        [128, DM], BF16, name=f"osb_{sc}", tag="osb"
                    )
                osb = ref["osb"]
                op_ps = psum.tile([128, 512], F32, name=f"o_{sc}_{m}", tag="pj")
                for p in range(NPAIR):
                    nc.tensor.matmul(
                        op_ps[:],
                        lhsT=at_sb[p][:, sc * 128:(sc + 1) * 128],
                        rhs=wo_sb[:, p, m * 512:(m + 1) * 512],
                        start=(p == 0),
                        stop=(p == NPAIR - 1),
                    )
                with nc.allow_low_precision(reason="bf16 out"):
                    nc.vector.tensor_copy(
                        out=osb[:, m * 512:(m + 1) * 512], in_=op_ps[:]
                    )
                if m == 1:
                    nc.sync.dma_start(out=o[sc * 128:(sc + 1) * 128, :], in_=osb[:])

            return [lambda: m_part(0), lambda: m_part(1)]

        def out_sc(sc):
            for f in out_fillers(sc):
                f()

        # ---- interleaved issue schedule ----
        # Only pair 0's first Q/K tile is issued up front (it gates the first
        # exp); everything else — remaining Q/K tiles, V tiles, output
        # chunks — is spread as fillers through the scores/exp stream,
        # placed after their DMA arrivals and before their AV deadlines.
        # AV accumulations run through a queue pumped in the PE bubbles.
        av_q = []     # AvStates pending steps, oldest first
        ripe = []     # stepped out; recip+broadcast issued, await finish_rb

        def pump(n):
            while n > 0 and av_q:
                s = av_q[0]
                if not s.can_step():
                    break
                alive = s.step()
                n -= 1
                if not alive:
                    s.finish_recip()
                    if ripe:
                        ripe.pop(0).finish_rb()
                    ripe.append(s)
                    av_q.pop(0)

        def vf(sc):
            """Whole V tile as one filler: the 12-matmul PSUM group must stay
            closed before any other pj-tag allocation (open groups in a
            reused slot would be clobbered)."""
            parts = v_fillers(sc)

            def run():
                for f in parts:
                    f()

            return run

        def qf(p, nt):
            return [lambda: qk_half(p, nt, 0), lambda: qk_half(p, nt, 1)]

        # filler lists per (j, h); deadlines: qt/kt tile nt before (j=nt, h=2p);
        # v(sc) before the AV chunk kc=sc is pumped; v tiles no earlier than
        # the wv DMA arrivals (~15us).
        FILL = {
            (0, 0): qf(1, 0),
            (0, 1): qf(2, 0),
            (0, 2): qf(3, 0) + [vf(0), vf(1)],
            (0, 3): [vf(2), vf(3)],
            (0, 4): qf(0, 1),
            (0, 6): qf(1, 1),
            (1, 0): [vf(4), vf(5), vf(6), vf(7)] + qf(2, 1),
            (1, 1): qf(3, 1),
            (1, 2): qf(0, 2),
            (1, 5): qf(1, 2),
            (2, 0): [vf(8), vf(9), vf(10), vf(11)] + qf(2, 2),
            (2, 1): qf(3, 2),
            (2, 2): qf(0, 3) + out_fillers(0),
            (2, 3): qf(1, 3) + out_fillers(1),
            (2, 4): qf(2, 3),
            (2, 5): qf(3, 3),
            (2, 6): [vf(12)],
            (2, 7): [vf(13)],
            (3, 0): [vf(14), vf(15)],
            (3, 2): out_fillers(2) + out_fillers(3),
            (3, 3): out_fillers(4) + out_fillers(5),
            (3, 4): out_fillers(6) + out_fillers(7),
            (3, 5): out_fillers(8) + out_fillers(9),
            (3, 6): out_fillers(10),
            (3, 7): out_fillers(11),
        }

        qk_tile(0, 0)
        for j in range(NQT):
            for h in range(8):
                fillers = list(FILL.get((j, h), []))
                rate = 3 if (j == 0 and h >= 3) or (j, h) in ((1, 0), (3, 7)) else 2
                if j == 0 and h < 3:
                    rate = 0
                pts = []
                av_q.append(AvState(h, j, pts))
                for pc in range(2 * j + 2):
                    pts.append(st_exp_pc(h, j, pc))
                    if fillers:
                        fillers.pop(0)()
                    pump(rate)
                for f in fillers:
                    f()
        while av_q:
            pump(4)
        while ripe:
            ripe.pop(0).finish_rb()
        for sc in range(12, 16):
            out_sc(sc)


def _masks_np():
    # tri[r, c] = 1 where k_local <= q_local (unmasked on the diagonal block)
    r = np.arange(128)[:, None]
    c = np.arange(128)[None, :]
    return (c >= r).astype(ml_dtypes.bfloat16)


def _hilo(a, s):
    """Same-scale fp8 hi/lo split: a ~ (hi + lo)/s with hi, lo fp8e4."""
    hi = np.asarray(a * s, ml_dtypes.float8_e4m3)
    lo = np.asarray((a * s - hi.astype(np.float32)), ml_dtypes.float8_e4m3)
    return np.ascontiguousarray(hi), np.ascontiguousarray(lo)


def make_in_maps(input, Wq, bq, Wk, bk, Wv, Wo):
    scale = np.float32(1.0 / np.sqrt(D_K))
    masks = _masks_np()
    input = np.asarray(input, np.float32)
    Wq = np.asarray(Wq, np.float32)
    Wk = np.asarray(Wk, np.float32)
    Wv = np.asarray(Wv, np.float32)
    Wo = np.asarray(Wo, np.float32)
    bq = np.asarray(bq, np.float32)
    bk = np.asarray(bk, np.float32)
    in_maps = []
    xsplit = []
    for b in range(BATCH):
        xhi, xlo = _hilo(input[b].T, SX)
        xsplit.append(np.ascontiguousarray(np.stack([xhi, xlo], axis=1)))
    for c in range(NCORES):
        b, g = divmod(c, 2)
        cols = slice(g * HV, (g + 1) * HV)
        wqh, _ = _hilo(Wq[:, cols] * (scale * SQ), SW)
        wkh, _ = _hilo(Wk[:, cols] * SQ, SW)
        wvh, wvl = _hilo(Wv[:, cols], SW)
        in_maps.append(
            {
                "x8": xsplit[b],
                "wqh": wqh,
                "wkh": wkh,
                "wvh": wvh, "wvl": wvl,
                "wo": np.ascontiguousarray(
                    np.asarray(Wo[g * HV:(g + 1) * HV, :], ml_dtypes.bfloat16)
                ),
                "bq": np.ascontiguousarray(bq[cols] * (scale * SQ)),
                "bk": np.ascontiguousarray(bk[cols] * SQ),
                "masks": masks,
            }
        )
    return in_maps


def _numpy_fallback(input, attn_mask, Wq, bq, Wk, bk, Wv, bv, Wo, bo):
    """Host fallback for non-causal masks (should not trigger in practice)."""
    x = np.asarray(input, np.float32)
    mask = np.asarray(attn_mask)
    B, S_, _ = x.shape
    scale = np.float32(1.0 / np.sqrt(D_K))
    out = np.empty((B, S_, D_MODEL), np.float32)
    for b in range(B):
        q = (x[b] @ Wq + bq).reshape(S_, N_HEAD, D_K)
        k = (x[b] @ Wk + bk).reshape(S_, N_HEAD, D_K)
        v = (x[b] @ Wv + bv).reshape(S_, N_HEAD, D_V)
        attn = np.empty((S_, N_HEAD, D_V), np.float32)
        for h in range(N_HEAD):
            score = (q[:, h] @ k[:, h].T) * scale
            score = np.where(mask, -np.inf, score)
            score -= score.max(axis=-1, keepdims=True)
            p = np.exp(score)
            p /= p.sum(axis=-1, keepdims=True)
            attn[:, h] = p @ v[:, h]
        out[b] = attn.reshape(S_, N_HEAD * D_V) @ Wo + bo
    return out


_CACHED_RUNNER = None


def _make_runner(nc):
    """Build the shard_map-jitted PJRT executor once; reuse across calls."""
    import jax
    from jax.sharding import Mesh, PartitionSpec
    from jax.experimental.shard_map import shard_map
    from concourse import bass2jax

    bass2jax.install_neuronx_cc_hook()
    partition_name = nc.partition_id_tensor.name if nc.partition_id_tensor else None
    in_names, out_names, out_avals, zero_outs = [], [], [], []
    for alloc in nc.m.functions[0].allocations:
        if not isinstance(alloc, mybir.MemoryLocationSet):
            continue
        name = alloc.memorylocations[0].name
        if alloc.kind == "ExternalInput":
            if name != partition_name:
                in_names.append(name)
        elif alloc.kind == "ExternalOutput":
            out_names.append(name)
            shape = tuple(alloc.tensor_shape)
            dtype = mybir.dt.np(alloc.dtype)
            out_avals.append(jax.core.ShapedArray(shape, dtype))
            zero_outs.append(np.zeros(shape, dtype))
    n_params = len(in_names)
    n_outs = len(out_avals)
    all_in_names = list(in_names) + list(out_names)
    if partition_name is not None:
        all_in_names.append(partition_name)

    def _body(*args):
        operands = list(args)
        if partition_name is not None:
            operands.append(bass2jax.partition_id_tensor())
        outs = bass2jax._bass_exec_p.bind(
            *operands,
            out_avals=tuple(out_avals),
            in_names=tuple(all_in_names),
            out_names=tuple(out_names),
            lowering_input_output_aliases=(),
            sim_require_finite=True,
            sim_require_nnan=True,
            nc=nc,
        )
        return tuple(outs)

    devices = jax.devices()[:NCORES]
    mesh = Mesh(np.asarray(devices), ("core",))
    sharded = jax.jit(
        shard_map(
            _body,
            mesh=mesh,
            in_specs=(PartitionSpec("core"),) * (n_params + n_outs),
            out_specs=(PartitionSpec("core"),) * n_outs,
            check_rep=False,
        ),
        donate_argnums=tuple(range(n_params, n_params + n_outs)),
        keep_unused=True,
    )

    def run(in_maps):
        concat_in = [
            np.concatenate(
                [np.asarray(in_maps[c][nm]) for c in range(NCORES)], axis=0
            )
            for nm in in_names
        ]
        concat_zeros = [
            np.zeros((NCORES * z.shape[0], *z.shape[1:]), z.dtype) for z in zero_outs
        ]
        out_arrs = sharded(*concat_in, *concat_zeros)
        return [
            {
                nm: np.asarray(out_arrs[i]).reshape(NCORES, *out_avals[i].shape)[c]
                for i, nm in enumerate(out_names)
            }
            for c in range(NCORES)
        ]

    return run


def kernel(input, attn_mask, Wq, bq, Wk, bk, Wv, bv, Wo, bo):
    causal = np.triu(np.ones((SEQ, SEQ), bool), k=1)
    if not np.array_equal(np.asarray(attn_mask), causal):
        return _numpy_fallback(input, attn_mask, Wq, bq, Wk, bk, Wv, bv, Wo, bo)

    global _CACHED_NC, _CACHED_RUNNER
    if _CACHED_NC is None:
        _CACHED_NC = _build_nc()

    in_maps = make_in_maps(input, Wq, bq, Wk, bk, Wv, Wo)
    try:
        if _CACHED_RUNNER is None:
            _CACHED_RUNNER = _make_runner(_CACHED_NC)
        outs = _CACHED_RUNNER(in_maps)
    except Exception:
        # jit-caching fast path failed (e.g. jax version skew) — use the
        # stock executor.
        _CACHED_RUNNER = None
        outs = bass_utils.run_bass_kernel_spmd(
            _CACHED_NC, in_maps, core_ids=list(range(NCORES))
        ).results

    corr = (
        np.asarray(bv, np.float32) @ np.asarray(Wo, np.float32)
        + np.asarray(bo, np.float32)
    ).astype(np.float32)
    out = np.empty((BATCH, SEQ, D_MODEL), np.float32)
    for b in range(BATCH):
        out[b] = (
            outs[2 * b]["o"].astype(np.float32)
            + outs[2 * b + 1]["o"].astype(np.float32)
            + corr[None, :]
        )
    return out


# revision 66
# speedup vs baseline: 1.0621x; 1.0162x over previous
"""Multi-head causal attention (B=4, S=2048, D=1024, H=16, dk=dv=64) on 8 NeuronCores.

Sharding: core c -> (batch b = c//2, head-group g = c%2 of 8 heads).
Each core computes Q/K/V projections for its batch restricted to its 8 heads,
causal softmax attention, and a partial output projection with its 512 rows of
Wo.  The host sums the two partials per batch and adds the constant correction
bv @ Wo + bo (bv passes through attention linearly because softmax rows sum
to 1).

Dtype strategy (keyed to the TRN2 matmul cost model: cost = out_free x
cycles_per_row; fp8 DoubleRow = 0.5 c/r with 2x128 contraction per
instruction, everything else 1.0):
  - QKV projections: x and W shipped from host as same-scale fp8e4 hi/lo
    pairs; 3-term (hi.hi + lo.hi + hi.lo) DoubleRow accumulation -> 0.75
    cycles per 128-contraction chunk instead of 1.0, with ~bf16 accuracy.
  - Scores: Q^T/K^T evicted to fp8e4 (x2 / x16 scales folded host-side)
    stored as [*, 2, S] with a zero second block, so a single DoubleRow
    matmul with d_k=64 contraction costs 0.5 c/r (the zero block contributes
    nothing and is free).
  - exp on ACT with scale=1/256 folding the score descale; bf16 P out.
  - AV v-major: au^T[65, 512] per (h, q-tile) with the ones row of V' as
    softmax denominator; bf16 rhs = exp output.  Big 512-col matmuls keep
    the PE sequencer (131 ns per Ldweights+Matmult pair in the cost model)
    off the critical path.
  - Normalization: reciprocal_approx_fast on the denominator row, broadcast
    across 64 partitions with a tiny f32r matmul, one tensor_tensor multiply
    into bf16 A^T.
  - Output projection and DMA in bf16.

Engine budget per core: ACT (exp, ~143us) is the bottleneck; PE ~155us of
issue interleaved so scores start ~10us in; DVE/Pool carry evictions,
reciprocals, masks and copies.
"""

import numpy as np
import ml_dtypes
from contextlib import ExitStack

import concourse.bass as bass
import concourse.mybir as mybir
import concourse.tile as tile
from concourse import bacc, bass_utils

N_HEAD, D_MODEL, D_K, D_V = 16, 1024, 64, 64
BATCH, SEQ = 4, 2048
NCORES = 8
S = SEQ
DM = D_MODEL
HV = 8 * D_V          # 512 local head-value columns per core
KC = DM // 128        # 8 d_model chunks
NPAIR = 4             # local head pairs
NQT = S // 512        # 4 q-tiles
F32 = mybir.dt.float32
BF16 = mybir.dt.bfloat16
F8 = mybir.dt.float8e4

SX = 16.0             # fp8 scale for x (hi and lo use the same scale)
SW = 8.0              # fp8 scale for projection weights
SQ = 16.0             # extra scale on the Q/K paths so fp8 eviction is exact
EVS = 1.0 / (SX * SW)          # psum -> Q/K/V descale
EXPS = 1.0 / (2.0 * SQ * 8.0)  # q8*k8 -> exp argument (incl. 1/sqrt(dk))

_CACHED_NC = None


def _build_nc(nbody=1, phases="ABC"):
    nc = bacc.Bacc("TRN2", target_bir_lowering=False, debug=False)

    x8 = nc.dram_tensor("x8", [DM, 2, S], F8, kind="ExternalInput").ap()
    wqh = nc.dram_tensor("wqh", [DM, HV], F8, kind="ExternalInput").ap()
    wkh = nc.dram_tensor("wkh", [DM, HV], F8, kind="ExternalInput").ap()
    wvh = nc.dram_tensor("wvh", [DM, HV], F8, kind="ExternalInput").ap()
    wvl = nc.dram_tensor("wvl", [DM, HV], F8, kind="ExternalInput").ap()
    wo = nc.dram_tensor("wo", [HV, DM], BF16, kind="ExternalInput").ap()
    bq = nc.dram_tensor("bq", [HV], F32, kind="ExternalInput").ap()
    bk = nc.dram_tensor("bk", [HV], F32, kind="ExternalInput").ap()
    masks = nc.dram_tensor("masks", [128, 128], BF16, kind="ExternalInput").ap()
    o = nc.dram_tensor("o", [S, DM], BF16, kind="ExternalOutput").ap()

    with tile.TileContext(nc) as tc:
        for _ in range(nbody):
            _build_kernel(tc, nc, x8, wqh, wkh, wvh, wvl,
                          wo, bq, bk, masks, o)
    nc.compile()
    return nc


def _build_kernel(tc, nc, x8, wqh, wkh, wvh, wvl,
                  wo, bq, bk, masks, o):
    EXP = mybir.ActivationFunctionType.Exp
    MULT = mybir.AluOpType.mult
    ADD = mybir.AluOpType.add

    with ExitStack() as ctx:
        pp = ctx.enter_context(tc.tile_pool(name="persist", bufs=1))

        # ---- persistent SBUF ----
        xhl = pp.tile([128, KC, 2, S], F8, name="xhl", tag="xhl")
        wq8 = pp.tile([128, KC, HV], F8, name="wq8", tag="wq8")
        wk8 = pp.tile([128, KC, HV], F8, name="wk8", tag="wk8")
        wv8 = [pp.tile([128, KC, HV], F8, name=f"wv8{i}", tag=f"wv8{i}") for i in range(2)]
        wo_sb = pp.tile([128, NPAIR, DM], BF16, name="wo_sb", tag="wo_sb")
        # Q^T/K^T per pair: [128, 2, S] fp8; block 1 stays zero (DoubleRow pad)
        qt8 = [pp.tile([128, 2, S], F8, name=f"qt8{p}", tag=f"qt8{p}") for p in range(NPAIR)]
        kt8 = [pp.tile([128, 2, S], F8, name=f"kt8{p}", tag=f"kt8{p}") for p in range(NPAIR)]
        # V' bf16: [128 kpos, s-chunk, head, 64+ones]
        vpr = pp.tile([128, S // 128, 8, 65], BF16, name="vpr", tag="vpr")
        at_sb = [pp.tile([128, S], BF16, name=f"at{p}", tag=f"at{p}") for p in range(NPAIR)]
        mask_sb = pp.tile([128, 128], BF16, name="mask_sb", tag="mask_sb")
        bq_sb = pp.tile([128, NPAIR], F32, name="bq_sb", tag="bq_sb")
        bk_sb = pp.tile([128, NPAIR], F32, name="bk_sb", tag="bk_sb")
        ones_sb = pp.tile([1, 64], mybir.dt.float32r, name="ones_sb", tag="ones_sb")

        psum = ctx.enter_context(tc.tile_pool(name="psum", bufs=2, space="PSUM"))
        # Banks: st 2x2 + au 2x1 + pj 2x1 = 8.

        # ---- zero pads / ones columns (before any use) ----
        for p in range(NPAIR):
            nc.gpsimd.memset(qt8[p][:, 1, :].bitcast(F32), 0.0)
            nc.gpsimd.memset(kt8[p][:, 1, :].bitcast(F32), 0.0)
        nc.gpsimd.memset(vpr[:, :, :, 64:65], 1.0)
        nc.gpsimd.memset(ones_sb[:].bitcast(F32), 1.0)

        # ---- DMA issue order: tiny first, then what phase A consumes first ----
        # First Q/K tile (q-cols 0:512) gates the exp pipeline: weights first
        # (so the PE chases the x chunks as they land), then its x columns,
        # then the rest streams in 512-col pieces.  x hi/lo are packed in one
        # dram tensor so each piece is one DMA.
        nc.sync.dma_start(out=wq8[:], in_=wqh.rearrange("(c p) m -> p c m", p=128))
        nc.sync.dma_start(out=wk8[:], in_=wkh.rearrange("(c p) m -> p c m", p=128))
        nc.sync.dma_start(out=bq_sb[:], in_=bq.rearrange("(pair r) -> r pair", r=128))
        nc.sync.dma_start(out=bk_sb[:], in_=bk.rearrange("(pair r) -> r pair", r=128))
        for kc in range(KC):
            nc.sync.dma_start(
                out=xhl[:, kc, :, 0:512], in_=x8[kc * 128:(kc + 1) * 128, :, 0:512]
            )
        nc.sync.dma_start(out=mask_sb[:], in_=masks)
        nc.sync.dma_start(out=wv8[0][:], in_=wvh.rearrange("(c p) m -> p c m", p=128))
        nc.sync.dma_start(out=wv8[1][:], in_=wvl.rearrange("(c p) m -> p c m", p=128))
        for s0 in range(512, S, 512):
            for kc in range(KC):
                nc.sync.dma_start(
                    out=xhl[:, kc, :, s0:s0 + 512],
                    in_=x8[kc * 128:(kc + 1) * 128, :, s0:s0 + 512],
                )
        nc.sync.dma_start(
            out=wo_sb[:],
            in_=wo.rearrange("(pair p) c -> p pair c", p=128),
        )

        pt_pool = ctx.enter_context(tc.tile_pool(name="pt", bufs=18))
        pr_pool = ctx.enter_context(tc.tile_pool(name="pr", bufs=4))
        rbs_pool = ctx.enter_context(tc.tile_pool(name="rbs", bufs=4))
        osb_pool = ctx.enter_context(tc.tile_pool(name="osb", bufs=3))

        F32R = mybir.dt.float32r

        def qk_half(p, nt, which, terms=1):
            """Project Q^T or K^T for pair p, q-tile nt -> fp8 eviction.

            1-term (x_hi @ W_hi): the fp8 eviction noise dominates the
            dropped correction terms.
            """
            qs = nt * 512
            last = terms * (KC // 2) - 1
            w8, t8, b_sb = ((wq8, qt8, bq_sb), (wk8, kt8, bk_sb))[which]
            ps = psum.tile([128, 512], F32, name=f"qk_{which}_{p}_{nt}", tag="pj")
            n = 0
            for xi in range(terms):
                for pc in range(KC // 2):
                    nc.tensor.matmul(
                        ps[:],
                        lhsT=w8[:, 2 * pc:2 * pc + 2, p * 128:(p + 1) * 128],
                        rhs=xhl[:, 2 * pc:2 * pc + 2, xi, qs:qs + 512],
                        start=(n == 0),
                        stop=(n == last),
                        perf_mode=mybir.MatmulPerfMode.DoubleRow,
                    )
                    n += 1
            with nc.allow_low_precision(reason="fp8 eviction is the design"):
                nc.vector.tensor_scalar(
                    out=t8[p][:, 0, qs:qs + 512],
                    in0=ps[:],
                    scalar1=EVS,
                    scalar2=b_sb[:, p:p + 1],
                    op0=MULT,
                    op1=ADD,
                )

        def qk_tile(p, nt, terms=1):
            qk_half(p, nt, 0, terms)
            qk_half(p, nt, 1, terms)

        def v_fillers(sc):
            """Project V for s-chunk sc -> bf16 V' with ones column.

            Returned as two issue-thunks so the 12-matmul group can be
            spread across scores/exp slots.
            """
            ref = {}
            terms = ((0, 0), (1, 0), (0, 1))

            def part(lo, hi, first, last):
                if first:
                    ref["ps"] = psum.tile([128, 512], F32, name=f"v_{sc}", tag="pj")
                ps = ref["ps"]
                for n in range(lo, hi):
                    xi, wi = terms[n // (KC // 2)]
                    pc = n % (KC // 2)
                    nc.tensor.matmul(
                        ps[:],
                        lhsT=xhl[:, 2 * pc:2 * pc + 2, xi, sc * 128:(sc + 1) * 128],
                        rhs=wv8[wi][:, 2 * pc:2 * pc + 2, :],
                        start=(n == 0),
                        stop=(n == 11),
                        perf_mode=mybir.MatmulPerfMode.DoubleRow,
                    )
                if last:
                    with nc.allow_low_precision(reason="bf16 V"):
                        nc.vector.tensor_scalar_mul(
                            out=vpr[:, sc, :, 0:64],
                            in0=ps[:].rearrange("p (h c) -> p h c", h=8),
                            scalar1=EVS,
                        )

            return [
                lambda: part(0, 6, True, False),
                lambda: part(6, 12, False, True),
            ]

        def st_exp_pc(h, j, pc):
            """One S^T pair-tile + exp (+ diag masks) -> pt [128, 2, 512]."""
            p, hp = divmod(h, 2)
            r0 = hp * 64
            vp = max(0, 128 * (2 * pc) - 512 * j)
            st = psum.tile([128, 1024], F32, name=f"st_{h}_{j}_{pc}", tag="st")
            st3 = st[:].rearrange("p (c q) -> p c q", c=2)
            for c in range(2):
                kc = 2 * pc + c
                nc.tensor.matmul(
                    st3[:, c, vp:512],
                    lhsT=kt8[p][r0:r0 + 64, :, kc * 128:(kc + 1) * 128],
                    rhs=qt8[p][r0:r0 + 64, :, j * 512 + vp:(j + 1) * 512],
                    start=True,
                    stop=True,
                    perf_mode=mybir.MatmulPerfMode.DoubleRow,
                )
            pt = pt_pool.tile([128, 2, 512], BF16, name=f"pt_{h}_{j}_{pc}", tag="pt")
            nc.scalar.activation(pt[:, :, vp:512], st3[:, :, vp:512], EXP, scale=EXPS)
            for c in range(2):
                kc = 2 * pc + c
                i = kc - 4 * j
                if i >= 0:  # diagonal chunk: triangular 0/1 mask
                    q0 = 128 * i
                    nc.vector.tensor_tensor(
                        out=pt[:, c, q0:q0 + 128],
                        in0=pt[:, c, q0:q0 + 128],
                        in1=mask_sb[:],
                        op=MULT,
                    )
            return pt

        class AvState:
            """v-major AV accumulation au^T[65, 512] for one (h, j), issued
            one k-chunk at a time so it can interleave with later heads'
            scores (fills the PE bubbles while ACT drains st tiles).  The
            PSUM tile is allocated lazily at the first step so queued states
            don't hold banks."""

            def __init__(self, h, j, pts):
                self.h, self.j, self.pts = h, j, pts
                self.nk = 4 * j + 4
                self.kc = 0
                self.au = None

            def can_step(self):
                return self.kc < self.nk and self.kc // 2 < len(self.pts)

            def step(self):
                if not self.can_step():
                    return self.kc < self.nk
                if self.au is None:
                    self.au = psum.tile(
                        [65, 512], F32, name=f"au_{self.h}_{self.j}", tag="au"
                    )
                kc = self.kc
                vc = max(0, 128 * kc - 512 * self.j)
                nc.tensor.matmul(
                    self.au[:, vc:512],
                    lhsT=vpr[:, kc, self.h, :],
                    rhs=self.pts[kc // 2][:, kc % 2, vc:512],
                    start=(kc == 0),
                    stop=(kc == self.nk - 1),
                )
                self.kc += 1
                return self.kc < self.nk

            def finish_recip(self):
                while self.kc < self.nk:
                    self.step()
                self.r_sb = pr_pool.tile(
                    [1, 512], F32R, name=f"r_{self.h}_{self.j}", tag="r"
                )
                with nc.allow_low_precision(reason="f32r out is bit-identical"):
                    nc.vector.reciprocal(out=self.r_sb[:], in_=self.au[64:65, :])
                self.rb_sb = rbs_pool.tile(
                    [64, 512], F32R, name=f"rbs_{self.h}_{self.j}", tag="rbs"
                )
                nc.gpsimd.partition_broadcast(self.rb_sb[:], self.r_sb[:])

            def finish_rb(self):
                h, j, au = self.h, self.j, self.au
                p, hp = divmod(h, 2)
                r0 = hp * 64
                with nc.allow_low_precision(reason="bf16 attn out"):
                    nc.vector.tensor_tensor(
                        out=at_sb[p][r0:r0 + 64, j * 512:(j + 1) * 512],
                        in0=au[0:64, :],
                        in1=self.rb_sb[:],
                        op=MULT,
                    )

        def out_fillers(sc):
            """Output projection for one 128-row s-chunk, as two issue-thunks."""
            ref = {}

            def m_part(m):
                if m == 0:
                    ref["osb"] = osb_pool.tile(
                        [128, DM], BF16, name=f"osb_{sc}", tag="osb"
                    )
                osb = ref["osb"]
                op_ps = psum.tile([128, 512], F32, name=f"o_{sc}_{m}", tag="pj")
                for p in range(NPAIR):
                    nc.tensor.matmul(
                        op_ps[:],
                        lhsT=at_sb[p][:, sc * 128:(sc + 1) * 128],
                        rhs=wo_sb[:, p, m * 512:(m + 1) * 512],
                        start=(p == 0),
                        stop=(p == NPAIR - 1),
                    )
                with nc.allow_low_precision(reason="bf16 out"):
                    nc.vector.tensor_copy(
                        out=osb[:, m * 512:(m + 1) * 512], in_=op_ps[:]
                    )
                if m == 1:
                    nc.sync.dma_start(out=o[sc * 128:(sc + 1) * 128, :], in_=osb[:])

            return [lambda: m_part(0), lambda: m_part(1)]

        def out_sc(sc):
            for f in out_fillers(sc):
                f()

        # ---- interleaved issue schedule ----
        # Only pair 0's first Q/K tile is issued up front (it gates the first
        # exp); everything else — remaining Q/K tiles, V tiles, output
        # chunks — is spread as fillers through the scores/exp stream,
        # placed after their DMA arrivals and before their AV deadlines.
        # AV accumulations run through a queue pumped in the PE bubbles.
        av_q = []     # AvStates pending steps, oldest first
        ripe = []     # stepped out; recip+broadcast issued, await finish_rb

        def pump(n, allow_last=False):
            while n > 0 and av_q:
                s = av_q[0]
                if s is av_q[-1] and not allow_last:
                    break  # keep one head of lag so AV never chases its exps
                if not s.can_step():
                    break
                alive = s.step()
                n -= 1
                if not alive:
                    s.finish_recip()
                    if ripe:
                        ripe.pop(0).finish_rb()
                    ripe.append(s)
                    av_q.pop(0)

        def vf(sc):
            """Whole V tile as one filler: the 12-matmul PSUM group must stay
            closed before any other pj-tag allocation (open groups in a
            reused slot would be clobbered)."""
            parts = v_fillers(sc)

            def run():
                for f in parts:
                    f()

            return run

        def qf(p, nt):
            return [lambda: qk_half(p, nt, 0), lambda: qk_half(p, nt, 1)]

        # filler lists per (j, h); deadlines: qt/kt tile nt before (j=nt, h=2p);
        # v(sc) before the AV chunk kc=sc is pumped; v tiles no earlier than
        # the wv DMA arrivals (~15us).
        FILL = {
            (0, 0): qf(1, 0),
            (0, 1): qf(2, 0),
            (0, 2): qf(3, 0) + [vf(0), vf(1)],
            (0, 3): [vf(2), vf(3)],
            (0, 4): qf(0, 1),
            (0, 6): qf(1, 1),
            (1, 0): [vf(4), vf(5), vf(6), vf(7)] + qf(2, 1),
            (1, 1): qf(3, 1),
            (1, 2): qf(0, 2),
            (1, 5): qf(1, 2),
            (2, 0): [vf(8), vf(9), vf(10), vf(11)] + qf(2, 2),
            (2, 1): qf(3, 2),
            (2, 2): qf(0, 3) + out_fillers(0),
            (2, 3): qf(1, 3) + out_fillers(1),
            (2, 4): qf(2, 3),
            (2, 5): qf(3, 3),
            (2, 6): [vf(12)],
            (2, 7): [vf(13)],
            (3, 0): [vf(14), vf(15)],
            (3, 2): out_fillers(2) + out_fillers(3),
            (3, 3): out_fillers(4) + out_fillers(5),
            (3, 4): out_fillers(6) + out_fillers(7),
            (3, 5): out_fillers(8) + out_fillers(9),
            (3, 6): out_fillers(10),
            (3, 7): out_fillers(11),
        }

        qk_tile(0, 0)
        for j in range(NQT):
            for h in range(8):
                fillers = list(FILL.get((j, h), []))
                rate = 3 if (j == 0 and h >= 3) or (j, h) in ((1, 0), (3, 7)) else 2
                if j == 0 and h < 3:
                    rate = 0
                pts = []
                av_q.append(AvState(h, j, pts))
                for pc in range(2 * j + 2):
                    pts.append(st_exp_pc(h, j, pc))
                    if fillers:
                        fillers.pop(0)()
                    pump(rate)
                for f in fillers:
                    f()
        while av_q:
            pump(4, allow_last=True)
        while ripe:
            ripe.pop(0).finish_rb()
        for sc in range(12, 16):
            out_sc(sc)


def _masks_np():
    # tri[r, c] = 1 where k_local <= q_local (unmasked on the diagonal block)
    r = np.arange(128)[:, None]
    c = np.arange(128)[None, :]
    return (c >= r).astype(ml_dtypes.bfloat16)


def _hilo(a, s):
    """Same-scale fp8 hi/lo split: a ~ (hi + lo)/s with hi, lo fp8e4."""
    hi = np.asarray(a * s, ml_dtypes.float8_e4m3)
    lo = np.asarray((a * s - hi.astype(np.float32)), ml_dtypes.float8_e4m3)
    return np.ascontiguousarray(hi), np.ascontiguousarray(lo)


def make_in_maps(input, Wq, bq, Wk, bk, Wv, Wo):
    scale = np.float32(1.0 / np.sqrt(D_K))
    masks = _masks_np()
    input = np.asarray(input, np.float32)
    Wq = np.asarray(Wq, np.float32)
    Wk = np.asarray(Wk, np.float32)
    Wv = np.asarray(Wv, np.float32)
    Wo = np.asarray(Wo, np.float32)
    bq = np.asarray(bq, np.float32)
    bk = np.asarray(bk, np.float32)
    in_maps = []
    xsplit = []
    for b in range(BATCH):
        xhi, xlo = _hilo(input[b].T, SX)
        xsplit.append(np.ascontiguousarray(np.stack([xhi, xlo], axis=1)))
    for c in range(NCORES):
        b, g = divmod(c, 2)
        cols = slice(g * HV, (g + 1) * HV)
        wqh, _ = _hilo(Wq[:, cols] * (scale * SQ), SW)
        wkh, _ = _hilo(Wk[:, cols] * SQ, SW)
        wvh, wvl = _hilo(Wv[:, cols], SW)
        in_maps.append(
            {
                "x8": xsplit[b],
                "wqh": wqh,
                "wkh": wkh,
                "wvh": wvh, "wvl": wvl,
                "wo": np.ascontiguousarray(
                    np.asarray(Wo[g * HV:(g + 1) * HV, :], ml_dtypes.bfloat16)
                ),
                "bq": np.ascontiguousarray(bq[cols] * (scale * SQ)),
                "bk": np.ascontiguousarray(bk[cols] * SQ),
                "masks": masks,
            }
        )
    return in_maps


def _numpy_fallback(input, attn_mask, Wq, bq, Wk, bk, Wv, bv, Wo, bo):
    """Host fallback for non-causal masks (should not trigger in practice)."""
    x = np.asarray(input, np.float32)
    mask = np.asarray(attn_mask)
    B, S_, _ = x.shape
    scale = np.float32(1.0 / np.sqrt(D_K))
    out = np.empty((B, S_, D_MODEL), np.float32)
    for b in range(B):
        q = (x[b] @ Wq + bq).reshape(S_, N_HEAD, D_K)
        k = (x[b] @ Wk + bk).reshape(S_, N_HEAD, D_K)
        v = (x[b] @ Wv + bv).reshape(S_, N_HEAD, D_V)
        attn = np.empty((S_, N_HEAD, D_V), np.float32)
        for h in range(N_HEAD):
            score = (q[:, h] @ k[:, h].T) * scale
            score = np.where(mask, -np.inf, score)
            score -= score.max(axis=-1, keepdims=True)
            p = np.exp(score)
            p /= p.sum(axis=-1, keepdims=True)
            attn[:, h] = p @ v[:, h]
        out[b] = attn.reshape(S_, N_HEAD * D_V) @ Wo + bo
    return out


_CACHED_RUNNER = None


def _make_runner(nc):
    """Build the shard_map-jitted PJRT executor once; reuse across calls."""
    import jax
    from jax.sharding import Mesh, PartitionSpec
    from jax.experimental.shard_map import shard_map
    from concourse import bass2jax

    bass2jax.install_neuronx_cc_hook()
    partition_name = nc.partition_id_tensor.name if nc.partition_id_tensor else None
    in_names, out_names, out_avals, zero_outs = [], [], [], []
    for alloc in nc.m.functions[0].allocations:
        if not isinstance(alloc, mybir.MemoryLocationSet):
            continue
        name = alloc.memorylocations[0].name
        if alloc.kind == "ExternalInput":
            if name != partition_name:
                in_names.append(name)
        elif alloc.kind == "ExternalOutput":
            out_names.append(name)
            shape = tuple(alloc.tensor_shape)
            dtype = mybir.dt.np(alloc.dtype)
            out_avals.append(jax.core.ShapedArray(shape, dtype))
            zero_outs.append(np.zeros(shape, dtype))
    n_params = len(in_names)
    n_outs = len(out_avals)
    all_in_names = list(in_names) + list(out_names)
    if partition_name is not None:
        all_in_names.append(partition_name)

    def _body(*args):
        operands = list(args)
        if partition_name is not None:
            operands.append(bass2jax.partition_id_tensor())
        outs = bass2jax._bass_exec_p.bind(
            *operands,
            out_avals=tuple(out_avals),
            in_names=tuple(all_in_names),
            out_names=tuple(out_names),
            lowering_input_output_aliases=(),
            sim_require_finite=True,
            sim_require_nnan=True,
            nc=nc,
        )
        return tuple(outs)

    devices = jax.devices()[:NCORES]
    mesh = Mesh(np.asarray(devices), ("core",))
    sharded = jax.jit(
        shard_map(
            _body,
            mesh=mesh,
            in_specs=(PartitionSpec("core"),) * (n_params + n_outs),
            out_specs=(PartitionSpec("core"),) * n_outs,
            check_rep=False,
        ),
        donate_argnums=tuple(range(n_params, n_params + n_outs)),
        keep_unused=True,
    )

    def run(in_maps):
        concat_in = [
            np.concatenate(
                [np.asarray(in_maps[c][nm]) for c in range(NCORES)], axis=0
            )
            for nm in in_names
        ]
        concat_zeros = [
            np.zeros((NCORES * z.shape[0], *z.shape[1:]), z.dtype) for z in zero_outs
        ]
        out_arrs = sharded(*concat_in, *concat_zeros)
        return [
            {
                nm: np.asarray(out_arrs[i]).reshape(NCORES, *out_avals[i].shape)[c]
                for i, nm in enumerate(out_names)
            }
            for c in range(NCORES)
        ]

    return run


def kernel(input, attn_mask, Wq, bq, Wk, bk, Wv, bv, Wo, bo):
    causal = np.triu(np.ones((SEQ, SEQ), bool), k=1)
    if not np.array_equal(np.asarray(attn_mask), causal):
        return _numpy_fallback(input, attn_mask, Wq, bq, Wk, bk, Wv, bv, Wo, bo)

    global _CACHED_NC, _CACHED_RUNNER
    if _CACHED_NC is None:
        _CACHED_NC = _build_nc()

    in_maps = make_in_maps(input, Wq, bq, Wk, bk, Wv, Wo)
    try:
        if _CACHED_RUNNER is None:
            _CACHED_RUNNER = _make_runner(_CACHED_NC)
        outs = _CACHED_RUNNER(in_maps)
    except Exception:
        # jit-caching fast path failed (e.g. jax version skew) — use the
        # stock executor.
        _CACHED_RUNNER = None
        outs = bass_utils.run_bass_kernel_spmd(
            _CACHED_NC, in_maps, core_ids=list(range(NCORES))
        ).results

    corr = (
        np.asarray(bv, np.float32) @ np.asarray(Wo, np.float32)
        + np.asarray(bo, np.float32)
    ).astype(np.float32)
    out = np.empty((BATCH, SEQ, D_MODEL), np.float32)
    for b in range(BATCH):
        out[b] = (
            outs[2 * b]["o"].astype(np.float32)
            + outs[2 * b + 1]["o"].astype(np.float32)
            + corr[None, :]
        )
    return out
